# revision 20
# baseline (speedup 1.0000x reference)
"""Trainium2 Bass kernel for nn_CrossAttention_5265629905601.

Reference computation (per batch b):
    q = query @ Wq.T + bq            [S, O]
    k = key   @ Wk.T + bk            [S, O]
    v = value @ Wv.T + bv            [S, O]
    scores = (q @ k.T) * O**-0.5     [S, S]
    probs  = softmax(scores, -1)
    out    = probs @ v               [S, O]

Sharding: data-parallel over batch — 16 batches / 8 cores = 2 per core.

Per-core strategy (algebraic factorization through DKV=768, fp8e4
DoubleRow on the softmax-damped Q side, bf16 on the V side):
  - Scores are factored through the SMALLER inner dim (DKV=768 < O=1024):
        scores[i,j] = (q_i@Wq.T+bq)·(k_j@Wk.T+bk)
                    = q_i M k_j^T + u k_j^T + (per-i const)      (M = Wq.T@Wk,
                                                                  u = bq@Wk)
    The per-i constant cancels in the softmax quotient, so the kernel
    computes e = exp(scale·(q~ @ k^T)) with q~ = q@M + 1⊗u.  No K
    projection at all; Q/K/scores work drops from S·DQ·O + S·DKV·O + S²·O
    to S·DQ·DKV + S²·DKV.  M, u host-precomputed in fp32, scaled x16 so
    fp8(M) clears the e4m3 subnormal cutoff (exp scale absorbs 1/16).
  - The whole Q side (q~ projection and score chains) runs in fp8e4
    perf_mode=DoubleRow: 2 fp8 weights/cell virtualize the PE to a
    256-deep contraction at ~1.08 cyc/col — 1.9x the bf16 stream rate.
    Quantization error injected BEFORE the softmax is damped by the 1/32
    score scale: measured end-to-end rel-err 1.27e-2 vs the 2e-2 budget
    (numpy-simulated 1.4e-2; all-bf16 was 2.9e-3).  DR layout: lhsT
    [128, 2, M] / rhs [128, 2, N] pairs of adjacent 128-chunks of the
    contraction dim; result = lhsT[:,0].T@rhs[:,0] + lhsT[:,1].T@rhs[:,1].
  - The V side (ZT = v^T E and out = ZT^T Wv^T) stays bf16: post-softmax
    operand quantization passes straight to the output (no damping) —
    fp8e4 there measures ~3e-2 total, over budget.
  - Activations are pre-transposed ON HOST to [D, S]; contraction dims
    land on partitions, no PE transposes anywhere.  q~T = M^T q^T with
    the u-bias applied as per-partition ACT bias during PSUM evacuation
    (ACT writes fp8 directly).  ZT lands transposed so the final
    projection needs no transposes either.
  - Max-subtraction skipped (scores ~ N(0, 0.33^2), exp never overflows).
  - Softmax denominator: DVE accumulates the 16 e-tiles, one ones-matmul
    crosses the partitions, DRAM-bounce transposes [1,512]->[128,4];
    1/colsum is applied as a per-partition ACT scale on the output tiles
    and bv is added by DVE from a host-broadcast [128, O] tile — the PE
    does NO rank-1 bias matmuls (removing the 64 rank-1 matmuls saved
    29 us/iter: a 1-partition-stationary matmul costs ~450 ns, not N/f).
  - Measured 432.9 us/iter on HW (from 611.7 us all-bf16 baseline).
    Effective PE clock ~2.0-2.15 GHz under sustained 8-core load (P0
    power throttle; nominal 2.4).  Per-MM stream rates measured on this
    part, 8 cores busy: bf16 N=512 ~254.8 ns; fp8-DR 256-contract
    ~240 ns with a reused stationary, ~264.9 ns when the stationary
    changes every MM (DR LDWEIGHTS loads 256 cols and only part-hides
    behind the ~230 ns stream); plain fp8 ~222 ns.
  - Negative results worth remembering: (1) restructuring the score
    phase kb-outer/qt-inner to reuse each DR stationary 4x (microbench
    says 225.9 ns/MM that way) made the FULL kernel 41 us SLOWER
    (473.9 us) — interleaving 4 accumulation chains mm-by-mm couples PE
    to the ACT exp evacuations and loses far more than the LDW saving,
    even though an evacuation-free microbench of the same pattern wins;
    keep chains contiguous per PSUM tile.  (2) splitting the weight DMA
    into stripes makes startup WORSE (~0.7 us SP trigger cost per
    dma_start dominates small transfers).  (3) fp8e4 on the V side:
    3.3e-2 rel-err, over budget — confirmed by numpy simulation.
"""

import numpy as np
import ml_dtypes
from contextlib import ExitStack

import concourse.bacc as bacc_mod
import concourse.tile as tile
import concourse.mybir as mybir
from concourse.bass_utils import run_bass_kernel_spmd

F32 = mybir.dt.float32
BF16 = mybir.dt.bfloat16
F32R = mybir.dt.float32r
FP8 = mybir.dt.float8e4
PM = mybir.MatmulPerfMode
AF = mybir.ActivationFunctionType
NP_BF16 = ml_dtypes.bfloat16
NP_FP8 = ml_dtypes.float8_e4m3
M_SCALE = 16.0                      # keeps fp8(M) out of the subnormal range

P = 128
N_CORES = 8
B_TOTAL, S, DQ, DKV, O = 16, 2048, 1024, 768, 1024
B_PER = B_TOTAL // N_CORES          # batches per core
SCALE = float(O) ** -0.5            # 1/32

S_TILES = S // 512                  # 4  (512-wide s tiles)
K_BLKS = S // P                     # 16 (128-row key blocks)
OC = O // P                         # 8  (128-wide output chunks)
DQC = DQ // P                       # 8  (query-feature 128-chunks)
DKC = DKV // P                      # 6  (key/value-feature 128-chunks)


def build_nc(n_reps: int = 1):
    """Build + compile the per-core Bass program.  n_reps>1 wraps the whole
    body in a runtime loop (used only for hardware timing)."""
    nc = bacc_mod.Bacc("TRN2", target_bir_lowering=False, debug=False,
                       num_devices=N_CORES)

    qT_in = nc.dram_tensor("qT_in", [B_PER, DQ, S], FP8, kind="ExternalInput")
    kT_in = nc.dram_tensor("kT_in", [B_PER, DKV, S], FP8, kind="ExternalInput")
    v_in = nc.dram_tensor("v_in", [B_PER, S, DKV], BF16, kind="ExternalInput")
    m_in = nc.dram_tensor("m_in", [DQ, DKV], FP8, kind="ExternalInput")
    wvt = nc.dram_tensor("wvt", [DKV, O], BF16, kind="ExternalInput")
    u_pp = nc.dram_tensor("u_pp", [P, DKC], F32, kind="ExternalInput")
    bv_bc = nc.dram_tensor("bv_bc", [P, O], BF16, kind="ExternalInput")
    ones_in = nc.dram_tensor("ones_in", [P, P], F32, kind="ExternalInput")
    out = nc.dram_tensor("out", [B_PER, S, O], F32, kind="ExternalOutput")

    with tile.TileContext(nc) as tc, ExitStack() as top:
        wpool = top.enter_context(tc.tile_pool(name="wpool", bufs=1))
        singles = top.enter_context(tc.tile_pool(name="singles", bufs=1))
        big = top.enter_context(tc.tile_pool(name="big", bufs=1))
        xin = top.enter_context(tc.tile_pool(name="xin", bufs=3))
        ztp = top.enter_context(tc.tile_pool(name="ztp", bufs=2))
        ep = top.enter_context(tc.tile_pool(name="ep", bufs=17))
        ost = top.enter_context(tc.tile_pool(name="ost", bufs=3))
        csl = top.enter_context(tc.tile_pool(name="csl", bufs=2))
        accp = top.enter_context(tc.tile_pool(name="accp", bufs=2))
        csd = top.enter_context(tc.tile_pool(name="csd", bufs=2, space="DRAM"))
        psMM = top.enter_context(tc.tile_pool(name="psMM", bufs=7, space="PSUM"))
        psCS = top.enter_context(tc.tile_pool(name="psCS", bufs=1, space="PSUM"))

        def body():
            # ---- per-iteration constant loads (weights, biases, ones) ----
            m_sb = wpool.tile([P, DQC, DKV], FP8, tag="m")
            nc.sync.dma_start(m_sb, m_in.rearrange("(dc p) v -> p dc v", p=P))
            u_sb = singles.tile([P, DKC], F32, tag="u")
            nc.sync.dma_start(u_sb, u_pp[:])
            # wv/ones/bv are first used ~100us in (attention phase); their
            # loads are deferred into b0/st1 so the critical first qin + m
            # transfers own the DMA queues at rep start
            wv_sb = wpool.tile([P, DKC, O], BF16, tag="wv")
            ones = singles.tile([P, P], F32R, tag="ones")
            bvb = singles.tile([P, O], BF16, tag="bvb")

            for b in range(B_PER):
                qt2 = big.tile([P, DKC, S], FP8, tag="qt2")
                kT_sb = big.tile([P, DKC, S], FP8, tag="kT")
                v_sb = big.tile([P, K_BLKS, DKV], BF16, tag="vsb")

                # ---------- q~ projection ----------
                for st in range(S_TILES):
                    sl = slice(st * 512, (st + 1) * 512)
                    qin = xin.tile([P, DQC, 512], FP8, tag="xin")
                    nc.sync.dma_start(
                        qin, qT_in[b].rearrange("(dc p) s -> p dc s", p=P)[:, :, sl])
                    # K/V bulk loads spread across the st iterations so the
                    # next qin never queues behind megabytes of K/V traffic
                    nc.sync.dma_start(
                        kT_sb[:, :, sl],
                        kT_in[b].rearrange("(dc p) s -> p dc s", p=P)[:, :, sl])
                    nc.sync.dma_start(
                        v_sb[:, st * 4:(st + 1) * 4, :],
                        v_in[b].rearrange("(kb p) v -> p kb v",
                                          p=P)[:, st * 4:(st + 1) * 4, :])
                    if b == 0 and st == 1:
                        nc.sync.dma_start(
                            wv_sb, wvt.rearrange("(dc p) o -> p dc o", p=P))
                        nc.sync.dma_start(ones, ones_in[:].bitcast(F32R))
                        nc.sync.dma_start(bvb, bv_bc[:])
                    for dv in range(DKC):
                        ps = psMM.tile([P, 512], F32, tag="mm")
                        for c in range(DQC // 2):
                            nc.tensor.matmul(
                                ps, m_sb[:, 2 * c:2 * c + 2, dv * P:(dv + 1) * P],
                                qin[:, 2 * c:2 * c + 2, :],
                                start=(c == 0), stop=(c == DQC // 2 - 1),
                                perf_mode=PM.DoubleRow)
                        nc.scalar.activation(qt2[:, dv, sl], ps, AF.Identity,
                                             bias=u_sb[:, dv:dv + 1])

                # ---------- attention ----------
                for qt in range(S_TILES):
                    qsl = slice(qt * 512, (qt + 1) * 512)
                    cs_ps = psCS.tile([1, 512], F32, tag="cs")
                    e_list = []
                    for kb in range(K_BLKS):
                        s_ps = psMM.tile([P, 512], F32, tag="mm")
                        for c in range(DKC // 2):
                            nc.tensor.matmul(
                                s_ps, kT_sb[:, 2 * c:2 * c + 2, kb * P:(kb + 1) * P],
                                qt2[:, 2 * c:2 * c + 2, qsl],
                                start=(c == 0), stop=(c == DKC // 2 - 1),
                                perf_mode=PM.DoubleRow)
                        e_t = ep.tile([P, 512], BF16, tag="E")
                        nc.scalar.activation(e_t, s_ps, AF.Exp,
                                             scale=SCALE / M_SCALE)
                        e_list.append(e_t)
                    # colsum: accumulate the 16 e-tiles on the (idle) DVE,
                    # then a single ones-matmul crosses the partitions — 1
                    # PE matmul per q-tile instead of 16
                    acc = accp.tile([P, 512], F32R, tag="acc")
                    nc.vector.tensor_add(acc, e_list[0], e_list[1])
                    for kb in range(2, K_BLKS):
                        acc2 = accp.tile([P, 512], F32R, tag="acc")
                        nc.vector.tensor_add(acc2, acc, e_list[kb])
                        acc = acc2
                    nc.tensor.matmul(cs_ps, ones[:, 0:1], acc,
                                     start=True, stop=True)
                    cs_sb = csl.tile([1, 512], F32, tag="cs_sb")
                    nc.vector.tensor_copy(cs_sb, cs_ps)
                    cs_d = csd.tile([512], F32, tag="csd")
                    nc.sync.dma_start(cs_d[:], cs_sb)
                    csT = csl.tile([P, 4], F32, tag="csT")
                    nc.sync.dma_start(csT, cs_d[:].rearrange("(j p) -> p j", p=P))
                    rcs = csl.tile([P, 4], F32, tag="rcs")
                    nc.vector.reciprocal(rcs, csT)
                    # ZT = v^T E — already-transposed weighted values
                    zt_sb = ztp.tile([P, DKC, 512], BF16, tag="zt")
                    for dvc in range(DKC):
                        z_ps = psMM.tile([P, 512], F32, tag="mm")
                        for kb in range(K_BLKS):
                            nc.tensor.matmul(
                                z_ps, v_sb[:, kb, dvc * P:(dvc + 1) * P],
                                e_list[kb],
                                start=(kb == 0), stop=(kb == K_BLKS - 1))
                        nc.vector.tensor_copy(zt_sb[:, dvc, :], z_ps)
                    # out = (ZT^T Wv^T + colsum (x) bv) * (1/colsum)
                    for qb in range(4):
                        for ot in range(2):
                            o_ps = psMM.tile([P, 512], F32, tag="mm")
                            for dvc in range(DKC):
                                nc.tensor.matmul(
                                    o_ps, zt_sb[:, dvc, qb * P:(qb + 1) * P],
                                    wv_sb[:, dvc, ot * 512:(ot + 1) * 512],
                                    start=(dvc == 0), stop=(dvc == DKC - 1))
                            # evacuate: ACT scales by 1/colsum (per-partition),
                            # DVE adds the broadcast bv row — no PE involved
                            o_t = ost.tile([P, 512], F32, tag="otmp")
                            nc.scalar.activation(o_t, o_ps, AF.Copy,
                                                 scale=rcs[:, qb:qb + 1])
                            o_sb = ost.tile([P, 512], F32, tag="osb")
                            nc.vector.tensor_add(
                                o_sb, o_t, bvb[:, ot * 512:(ot + 1) * 512])
                            nc.sync.dma_start(
                                out[b,
                                    qt * 512 + qb * P: qt * 512 + (qb + 1) * P,
                                    ot * 512:(ot + 1) * 512],
                                o_sb)

        if n_reps > 1:
            with tc.For_i(0, n_reps, staggered_reset=True):
                body()
        else:
            body()

    nc.compile()
    return nc


_nc_cache = {}


def _get_nc(n_reps: int = 1):
    if n_reps not in _nc_cache:
        _nc_cache[n_reps] = build_nc(n_reps)
    return _nc_cache[n_reps]


def make_in_maps(query, key, value, Wq, bq, Wk, bk, Wv, bv):
    """Host-side prep: shard activations over batch; transpose activations
    to [D, S]; precompute M = Wq.T@Wk and u = bq@Wk; cast to bf16."""
    qT = np.ascontiguousarray(
        np.asarray(query, np.float32).transpose(0, 2, 1)).astype(NP_FP8)
    kT = np.ascontiguousarray(
        np.asarray(key, np.float32).transpose(0, 2, 1)).astype(NP_FP8)
    vn = np.ascontiguousarray(np.asarray(value, np.float32)).astype(NP_BF16)
    Wq = np.asarray(Wq, np.float32)
    Wk = np.asarray(Wk, np.float32)
    M = Wq.T @ Wk                                   # [DQ, DKV], fp32
    u = np.asarray(bq, np.float32) @ Wk             # [DKV]
    shared = {
        "m_in": np.ascontiguousarray(M * M_SCALE).astype(NP_FP8),
        "wvt": np.ascontiguousarray(np.asarray(Wv, np.float32).T).astype(NP_BF16),
        "u_pp": np.ascontiguousarray((u * M_SCALE).reshape(DKC, P).T),
        "bv_bc": np.broadcast_to(
            np.asarray(bv, np.float32).astype(NP_BF16), (P, O)).copy(),
        "ones_in": np.ones((P, P), dtype=np.float32),
    }
    in_maps = []
    for c in range(N_CORES):
        sl = slice(c * B_PER, (c + 1) * B_PER)
        in_maps.append({
            "qT_in": qT[sl], "kT_in": kT[sl], "v_in": vn[sl], **shared,
        })
    return in_maps


def kernel(query, key, value, Wq, bq, Wk, bk, Wv, bv):
    in_maps = make_in_maps(query, key, value, Wq, bq, Wk, bk, Wv, bv)
    nc = _get_nc(1)
    res = run_bass_kernel_spmd(nc, in_maps, core_ids=list(range(N_CORES)))
    return np.concatenate([r["out"] for r in res.results], axis=0)



# revision 39
# speedup vs baseline: 1.0629x; 1.0629x over previous
"""Trainium2 Bass kernel for nn_CrossAttention_5265629905601.

Reference computation (per batch b):
    q = query @ Wq.T + bq            [S, O]
    k = key   @ Wk.T + bk            [S, O]
    v = value @ Wv.T + bv            [S, O]
    scores = (q @ k.T) * O**-0.5     [S, S]
    probs  = softmax(scores, -1)
    out    = probs @ v               [S, O]

Sharding: data-parallel over batch — 16 batches / 8 cores = 2 per core.

Per-core strategy (algebraic factorization through DKV=768, fp8e4
DoubleRow on the softmax-damped Q side, bf16 on the V side):
  - Scores are factored through the SMALLER inner dim (DKV=768 < O=1024):
        scores[i,j] = (q_i@Wq.T+bq)·(k_j@Wk.T+bk)
                    = q_i M k_j^T + u k_j^T + (per-i const)      (M = Wq.T@Wk,
                                                                  u = bq@Wk)
    The per-i constant cancels in the softmax quotient, so the kernel
    computes e = exp(scale·(q~ @ k^T)) with q~ = q@M + 1⊗u.  No K
    projection at all; Q/K/scores work drops from S·DQ·O + S·DKV·O + S²·O
    to S·DQ·DKV + S²·DKV.  M, u host-precomputed in fp32, scaled x16 so
    fp8(M) clears the e4m3 subnormal cutoff (exp scale absorbs 1/16).
  - The whole Q side (q~ projection and score chains) runs in fp8e4
    perf_mode=DoubleRow: 2 fp8 weights/cell virtualize the PE to a
    256-deep contraction at ~1.08 cyc/col — 1.9x the bf16 stream rate.
    Quantization error injected BEFORE the softmax is damped by the 1/32
    score scale: measured end-to-end rel-err 1.27e-2 vs the 2e-2 budget
    (numpy-simulated 1.4e-2; all-bf16 was 2.9e-3).  DR layout: lhsT
    [128, 2, M] / rhs [128, 2, N] pairs of adjacent 128-chunks of the
    contraction dim; result = lhsT[:,0].T@rhs[:,0] + lhsT[:,1].T@rhs[:,1].
  - The V side (ZT = v^T E and out = ZT^T Wv^T) stays bf16: post-softmax
    operand quantization passes straight to the output (no damping) —
    fp8e4 there measures ~3e-2 total, over budget.
  - Activations are pre-transposed ON HOST to [D, S]; contraction dims
    land on partitions, no PE transposes anywhere.  q~T = M^T q^T with
    the u-bias applied as per-partition ACT bias during PSUM evacuation
    (ACT writes fp8 directly).  ZT lands transposed so the final
    projection needs no transposes either.
  - Max-subtraction skipped (scores ~ N(0, 0.33^2), exp never overflows).
  - Softmax denominator: DVE accumulates the 16 e-tiles, one ones-matmul
    crosses the partitions, DRAM-bounce transposes [1,512]->[128,4];
    1/colsum is applied as a per-partition ACT scale on the output tiles
    and bv is added by DVE from a host-broadcast [128, O] tile — the PE
    does NO rank-1 bias matmuls (removing the 64 rank-1 matmuls saved
    29 us/iter: a 1-partition-stationary matmul costs ~450 ns, not N/f).
  - Measured 432.9 us/iter on HW (from 611.7 us all-bf16 baseline).
    Effective PE clock ~2.0-2.15 GHz under sustained 8-core load (P0
    power throttle; nominal 2.4).  Per-MM stream rates measured on this
    part, 8 cores busy: bf16 N=512 ~254.8 ns; fp8-DR 256-contract
    ~240 ns with a reused stationary, ~264.9 ns when the stationary
    changes every MM (DR LDWEIGHTS loads 256 cols and only part-hides
    behind the ~230 ns stream); plain fp8 ~222 ns.
  - Negative results worth remembering: (1) restructuring the score
    phase kb-outer/qt-inner to reuse each DR stationary 4x (microbench
    says 225.9 ns/MM that way) made the FULL kernel 41 us SLOWER
    (473.9 us) — interleaving 4 accumulation chains mm-by-mm couples PE
    to the ACT exp evacuations and loses far more than the LDW saving,
    even though an evacuation-free microbench of the same pattern wins;
    keep chains contiguous per PSUM tile.  (2) splitting the weight DMA
    into stripes makes startup WORSE (~0.7 us SP trigger cost per
    dma_start dominates small transfers).  (3) fp8e4 on the V side:
    3.3e-2 rel-err, over budget — confirmed by numpy simulation.
    (4) PARTIAL fp8 ZT (last 4 of 16 key blocks as one DoubleRow pair
    chain, est. +1.05e-2 err in quadrature, ~23 us/iter win) could not
    be brought up: a DR matmul accumulating into the same PSUM group as
    bf16 matmuls crashed the device (NRT_EXEC_UNIT_UNRECOVERABLE), and
    the separate-chain variant merged by a DVE tensor_add reading TWO
    PSUM operands crashed walrus codegen (C++ throw in a backend pass).
    If retried: evacuate the fp8 partial chain through ACT to SBUF
    first, then a single-PSUM-operand DVE add.
"""

import numpy as np
import ml_dtypes
from contextlib import ExitStack

import concourse.bacc as bacc_mod
import concourse.tile as tile
import concourse.mybir as mybir
from concourse.bass_utils import run_bass_kernel_spmd

F32 = mybir.dt.float32
BF16 = mybir.dt.bfloat16
F32R = mybir.dt.float32r
FP8 = mybir.dt.float8e4
PM = mybir.MatmulPerfMode
AF = mybir.ActivationFunctionType
NP_BF16 = ml_dtypes.bfloat16
NP_FP8 = ml_dtypes.float8_e4m3
M_SCALE = 16.0                      # keeps fp8(M) out of the subnormal range

P = 128
N_CORES = 8
B_TOTAL, S, DQ, DKV, O = 16, 2048, 1024, 768, 1024
B_PER = B_TOTAL // N_CORES          # batches per core
SCALE = float(O) ** -0.5            # 1/32

S_TILES = S // 512                  # 4  (512-wide s tiles)
K_BLKS = S // P                     # 16 (128-row key blocks)
OC = O // P                         # 8  (128-wide output chunks)
DQC = DQ // P                       # 8  (query-feature 128-chunks)
DKC = DKV // P                      # 6  (key/value-feature 128-chunks)


def build_nc(n_reps: int = 1):
    """Build + compile the per-core Bass program.  n_reps>1 wraps the whole
    body in a runtime loop (used only for hardware timing)."""
    nc = bacc_mod.Bacc("TRN2", target_bir_lowering=False, debug=False,
                       num_devices=N_CORES)

    qT_in = nc.dram_tensor("qT_in", [B_PER, DQ, S], FP8, kind="ExternalInput")
    kT_in = nc.dram_tensor("kT_in", [B_PER, DKV, S], FP8, kind="ExternalInput")
    v_in = nc.dram_tensor("v_in", [B_PER, S, DKV], BF16, kind="ExternalInput")
    m_in = nc.dram_tensor("m_in", [DQ, DKV], FP8, kind="ExternalInput")
    wvt = nc.dram_tensor("wvt", [DKV, O], BF16, kind="ExternalInput")
    u_pp = nc.dram_tensor("u_pp", [P, DKC], F32, kind="ExternalInput")
    bv_bc = nc.dram_tensor("bv_bc", [P, O], BF16, kind="ExternalInput")
    ones_in = nc.dram_tensor("ones_in", [P, P], F32, kind="ExternalInput")
    out = nc.dram_tensor("out", [B_PER, S, O], F32, kind="ExternalOutput")

    with tile.TileContext(nc) as tc, ExitStack() as top:
        wpool = top.enter_context(tc.tile_pool(name="wpool", bufs=1))
        singles = top.enter_context(tc.tile_pool(name="singles", bufs=1))
        big = top.enter_context(tc.tile_pool(name="big", bufs=1))
        xin = top.enter_context(tc.tile_pool(name="xin", bufs=3))
        ztp = top.enter_context(tc.tile_pool(name="ztp", bufs=2))
        ep = top.enter_context(tc.tile_pool(name="ep", bufs=17))
        ost = top.enter_context(tc.tile_pool(name="ost", bufs=3))
        csl = top.enter_context(tc.tile_pool(name="csl", bufs=2))
        accp = top.enter_context(tc.tile_pool(name="accp", bufs=2))
        csd = top.enter_context(tc.tile_pool(name="csd", bufs=2, space="DRAM"))
        psMM = top.enter_context(tc.tile_pool(name="psMM", bufs=7, space="PSUM"))
        psCS = top.enter_context(tc.tile_pool(name="psCS", bufs=1, space="PSUM"))

        def body():
            # ---- per-iteration constant loads (weights, biases, ones) ----
            m_sb = wpool.tile([P, DQC, DKV], FP8, tag="m")
            nc.sync.dma_start(m_sb, m_in.rearrange("(dc p) v -> p dc v", p=P))
            u_sb = singles.tile([P, DKC], F32, tag="u")
            nc.sync.dma_start(u_sb, u_pp[:])
            # wv/ones/bv are first used ~100us in (attention phase); their
            # loads are deferred into b0/st1 so the critical first qin + m
            # transfers own the DMA queues at rep start
            wv_sb = wpool.tile([P, DKC, O], BF16, tag="wv")
            ones = singles.tile([P, P], F32R, tag="ones")
            bvb = singles.tile([P, O], BF16, tag="bvb")

            for b in range(B_PER):
                qt2 = big.tile([P, DKC, S], FP8, tag="qt2")
                kT_sb = big.tile([P, DKC, S], FP8, tag="kT")
                v_sb = big.tile([P, K_BLKS, DKV], BF16, tag="vsb")

                # ---------- q~ projection ----------
                for st in range(S_TILES):
                    sl = slice(st * 512, (st + 1) * 512)
                    qin = xin.tile([P, DQC, 512], FP8, tag="xin")
                    nc.sync.dma_start(
                        qin, qT_in[b].rearrange("(dc p) s -> p dc s", p=P)[:, :, sl])
                    # K/V bulk loads spread across the st iterations so the
                    # next qin never queues behind megabytes of K/V traffic
                    nc.sync.dma_start(
                        kT_sb[:, :, sl],
                        kT_in[b].rearrange("(dc p) s -> p dc s", p=P)[:, :, sl])
                    nc.sync.dma_start(
                        v_sb[:, st * 4:(st + 1) * 4, :],
                        v_in[b].rearrange("(kb p) v -> p kb v",
                                          p=P)[:, st * 4:(st + 1) * 4, :])
                    if b == 0 and st == 1:
                        nc.sync.dma_start(
                            wv_sb, wvt.rearrange("(dc p) o -> p dc o", p=P))
                        nc.sync.dma_start(ones, ones_in[:].bitcast(F32R))
                        nc.sync.dma_start(bvb, bv_bc[:])
                    for dv in range(DKC):
                        ps = psMM.tile([P, 512], F32, tag="mm")
                        for c in range(DQC // 2):
                            nc.tensor.matmul(
                                ps, m_sb[:, 2 * c:2 * c + 2, dv * P:(dv + 1) * P],
                                qin[:, 2 * c:2 * c + 2, :],
                                start=(c == 0), stop=(c == DQC // 2 - 1),
                                perf_mode=PM.DoubleRow)
                        nc.scalar.activation(qt2[:, dv, sl], ps, AF.Identity,
                                             bias=u_sb[:, dv:dv + 1])

                # ---------- attention ----------
                for qt in range(S_TILES):
                    qsl = slice(qt * 512, (qt + 1) * 512)
                    cs_ps = psCS.tile([1, 512], F32, tag="cs")
                    e_list = []
                    for kb in range(K_BLKS):
                        s_ps = psMM.tile([P, 512], F32, tag="mm")
                        for c in range(DKC // 2):
                            nc.tensor.matmul(
                                s_ps, kT_sb[:, 2 * c:2 * c + 2, kb * P:(kb + 1) * P],
                                qt2[:, 2 * c:2 * c + 2, qsl],
                                start=(c == 0), stop=(c == DKC // 2 - 1),
                                perf_mode=PM.DoubleRow)
                        e_t = ep.tile([P, 512], BF16, tag="E")
                        nc.scalar.activation(e_t, s_ps, AF.Exp,
                                             scale=SCALE / M_SCALE)
                        e_list.append(e_t)
                    # colsum: accumulate the 16 e-tiles on the (idle) DVE,
                    # then a single ones-matmul crosses the partitions — 1
                    # PE matmul per q-tile instead of 16
                    acc = accp.tile([P, 512], F32R, tag="acc")
                    nc.vector.tensor_add(acc, e_list[0], e_list[1])
                    for kb in range(2, K_BLKS):
                        acc2 = accp.tile([P, 512], F32R, tag="acc")
                        nc.vector.tensor_add(acc2, acc, e_list[kb])
                        acc = acc2
                    nc.tensor.matmul(cs_ps, ones[:, 0:1], acc,
                                     start=True, stop=True)
                    cs_sb = csl.tile([1, 512], F32, tag="cs_sb")
                    nc.vector.tensor_copy(cs_sb, cs_ps)
                    cs_d = csd.tile([512], F32, tag="csd")
                    nc.sync.dma_start(cs_d[:], cs_sb)
                    csT = csl.tile([P, 4], F32, tag="csT")
                    nc.sync.dma_start(csT, cs_d[:].rearrange("(j p) -> p j", p=P))
                    rcs = csl.tile([P, 4], F32, tag="rcs")
                    nc.vector.reciprocal(rcs, csT)
                    # ZT = v^T E — already-transposed weighted values
                    zt_sb = ztp.tile([P, DKC, 512], BF16, tag="zt")
                    for dvc in range(DKC):
                        z_ps = psMM.tile([P, 512], F32, tag="mm")
                        for kb in range(K_BLKS):
                            nc.tensor.matmul(
                                z_ps, v_sb[:, kb, dvc * P:(dvc + 1) * P],
                                e_list[kb],
                                start=(kb == 0), stop=(kb == K_BLKS - 1))
                        nc.vector.tensor_copy(zt_sb[:, dvc, :], z_ps)
                    # out = (ZT^T Wv^T + colsum (x) bv) * (1/colsum)
                    for qb in range(4):
                        for ot in range(2):
                            o_ps = psMM.tile([P, 512], F32, tag="mm")
                            for dvc in range(DKC):
                                nc.tensor.matmul(
                                    o_ps, zt_sb[:, dvc, qb * P:(qb + 1) * P],
                                    wv_sb[:, dvc, ot * 512:(ot + 1) * 512],
                                    start=(dvc == 0), stop=(dvc == DKC - 1))
                            # evacuate: ACT scales by 1/colsum (per-partition),
                            # DVE adds the broadcast bv row — no PE involved
                            o_t = ost.tile([P, 512], F32, tag="otmp")
                            nc.scalar.activation(o_t, o_ps, AF.Copy,
                                                 scale=rcs[:, qb:qb + 1])
                            o_sb = ost.tile([P, 512], F32, tag="osb")
                            nc.vector.tensor_add(
                                o_sb, o_t, bvb[:, ot * 512:(ot + 1) * 512])
                            nc.sync.dma_start(
                                out[b,
                                    qt * 512 + qb * P: qt * 512 + (qb + 1) * P,
                                    ot * 512:(ot + 1) * 512],
                                o_sb)

        if n_reps > 1:
            with tc.For_i(0, n_reps, staggered_reset=True):
                body()
        else:
            body()

    nc.compile()
    return nc


_nc_cache = {}


def _get_nc(n_reps: int = 1):
    if n_reps not in _nc_cache:
        _nc_cache[n_reps] = build_nc(n_reps)
    return _nc_cache[n_reps]


def make_in_maps(query, key, value, Wq, bq, Wk, bk, Wv, bv):
    """Host-side prep: shard activations over batch; transpose activations
    to [D, S]; precompute M = Wq.T@Wk and u = bq@Wk; cast to bf16."""
    qT = np.ascontiguousarray(
        np.asarray(query, np.float32).transpose(0, 2, 1)).astype(NP_FP8)
    kT = np.ascontiguousarray(
        np.asarray(key, np.float32).transpose(0, 2, 1)).astype(NP_FP8)
    vn = np.ascontiguousarray(np.asarray(value, np.float32)).astype(NP_BF16)
    Wq = np.asarray(Wq, np.float32)
    Wk = np.asarray(Wk, np.float32)
    M = Wq.T @ Wk                                   # [DQ, DKV], fp32
    u = np.asarray(bq, np.float32) @ Wk             # [DKV]
    shared = {
        "m_in": np.ascontiguousarray(M * M_SCALE).astype(NP_FP8),
        "wvt": np.ascontiguousarray(np.asarray(Wv, np.float32).T).astype(NP_BF16),
        "u_pp": np.ascontiguousarray((u * M_SCALE).reshape(DKC, P).T),
        "bv_bc": np.broadcast_to(
            np.asarray(bv, np.float32).astype(NP_BF16), (P, O)).copy(),
        "ones_in": np.ones((P, P), dtype=np.float32),
    }
    in_maps = []
    for c in range(N_CORES):
        sl = slice(c * B_PER, (c + 1) * B_PER)
        in_maps.append({
            "qT_in": qT[sl], "kT_in": kT[sl], "v_in": vn[sl], **shared,
        })
    return in_maps


def kernel(query, key, value, Wq, bq, Wk, bk, Wv, bv):
    in_maps = make_in_maps(query, key, value, Wq, bq, Wk, bk, Wv, bv)
    nc = _get_nc(1)
    res = run_bass_kernel_spmd(nc, in_maps, core_ids=list(range(N_CORES)))
    return np.concatenate([r["out"] for r in res.results], axis=0)

